# revision 1
# baseline (speedup 1.0000x reference)
"""Trainium2 Bass kernel for CropConv: 3x3 same-padding conv (64->64 ch) on
[16, 64, 128, 128] fp32 input, with a static crop mask zeroing output rows/cols
[44:84).

Strategy (data-parallel over batch, 8 cores x 2 images each):
  - Host marshals x into a zero-padded row-major layout with row stride 129
    (131 padded rows: top pad, bottom pad, stream slack; the left zero column
    of each row doubles as the previous row's right pad), so every conv tap
    (kh, kw) of an output row-chunk is one contiguous rhs slice.
  - Per core, image 0 lives in SBUF partitions 0-63 (partition = in-channel),
    image 1 in partitions 64-127.
  - The conv is 9 PSUM-accumulated TensorE matmuls per output chunk:
    out[oc, pix] += W[kh,kw][ic, oc].T @ x[ic, shifted pix].  K = M = 64, so
    four matmuls run concurrently in the four 64x64 quadrants of the PE array
    (row-half = image, col-half = chunk pairing (c, c+22)), in fp32r mode.
  - PSUM -> SBUF stage copy, crop-mask memsets on the stage, then large
    row-contiguous DMA stores (full rows; mask zeroed on-chip).
"""

import numpy as np

# ---- problem constants (hardcoded; kernel.py must be self-contained) ----
B, C, H, W = 16, 64, 128, 128
OC, KS = 64, 3
N_CORES = 8
IMGS = B // N_CORES  # 2 images per core

WP = W + 1            # padded row stride: 129
HP = H + 3            # padded rows in the x buffer: 131
XLEN = HP * WP        # 16899 fp32 per partition

RPC = 3               # output rows per chunk
NCH = (H + RPC - 1) // RPC          # 43 chunks per image (last has 2 rows)
NPAIR = 21            # chunk pairs (c, c+22); chunk 21 is the leftover
CHN = RPC * WP        # matmul free dim per full chunk: 387
CHS = RPC * W         # compact stage slot stride: 384
STLEN = 2 * 22 * CHS  # stage free size: 16896 (= 132 rows * 128)

CROP0, CROP1 = 44, 84  # masked rows/cols [44, 84)

_CACHE = {}


def _build_module():
    import concourse.tile as tile
    from concourse import bacc, mybir

    f32 = mybir.dt.float32
    bf16 = mybir.dt.bfloat16

    nc = bacc.Bacc("TRN2", target_bir_lowering=False, debug=False,
                   num_devices=N_CORES)

    x_ap = nc.dram_tensor("xin", [IMGS, C, XLEN], bf16,
                          kind="ExternalInput").ap()
    w_ap = nc.dram_tensor("wt", [C, KS * KS, OC], bf16,
                          kind="ExternalInput").ap()
    y_ap = nc.dram_tensor("yout", [IMGS, OC, H, W], f32,
                          kind="ExternalOutput").ap()

    x_bc = x_ap.rearrange("b c l -> (b c) l")  # [128, XLEN]

    with tile.TileContext(nc) as tc:
        with tc.tile_pool(name="big", bufs=1) as big, \
             tc.tile_pool(name="psum", bufs=8, space="PSUM") as pp:

            x_sb = big.tile([128, XLEN], bf16, tag="xbuf")
            stage = big.tile([128, STLEN], f32, tag="stage")
            w_sb = big.tile([128, KS * KS * OC], bf16, tag="wbuf")

            st3 = stage.rearrange("p (h w) -> p h w", w=W)    # [128, 132, 128]

            # weights, replicated into both partition halves
            w_flat = w_ap.rearrange("i t o -> i (t o)")
            nc.sync.dma_start(out=w_sb[0:64, :], in_=w_flat)
            nc.sync.dma_start(out=w_sb[64:128, :], in_=w_flat)

            # x loads: contiguous padded-row segments, upper-half-first
            # interleave so both chunk-pair halves become computable early
            segs = [(65, 99), (0, 33), (99, 131), (33, 65)]
            for (a, b_) in segs:
                nc.sync.dma_start(out=x_sb[:, a * WP:b_ * WP],
                                  in_=x_bc[:, a * WP:b_ * WP])

            def lhsT(half, t):
                return w_sb[half * 64:(half + 1) * 64, t * OC:(t + 1) * OC]

            def rhs(half, c, kh, kw, n):
                off = (RPC * c + kh) * WP + kw
                return x_sb[half * 64:(half + 1) * 64, off:off + n]

            def chunk_n(c):
                return 2 * WP if c == NCH - 1 else CHN  # 258 for chunk 42

            store_plan = []  # (emit_after_pair, fn)

            def emit_stores_ready(done_pairs):
                for item in list(store_plan):
                    if item[0] <= done_pairs:
                        item[1]()
                        store_plan.remove(item)

            # store pieces: (partition half, view-row range, img, y row range)
            # lower half: img0 view rows 0..65 -> y rows 0..65
            #             img1 view rows 66..131 -> y rows 0..65
            # upper half: img0 view rows 0..61 -> y rows 66..127
            #             img1 view rows 66..127 -> y rows 66..127
            st4 = stage.rearrange("p (i h w) -> p i h w", i=2, w=W)

            def mk_store(half, r0, yr0, nrows):
                def go():
                    src = st4[half * 64:(half + 1) * 64, :, r0:r0 + nrows, :]
                    dst = y_ap[:, :, yr0:yr0 + nrows, :].rearrange(
                        "b o h w -> o b h w")
                    nc.scalar.dma_start(out=dst, in_=src)
                return go

            # ready_pair: pair index after which all needed slots are written.
            # view rows [vr0, vr0+nr) need pairs up to (vr0+nr-1)//3 for both
            # halves/images; lower rows beyond 63 additionally need the
            # leftover chunk 21 (pair index NPAIR+1).
            for (half, base_yr, tot) in [(0, 0, 66), (1, 66, 62)]:
                for pr0 in range(0, tot, 33):
                    nr = min(33, tot - pr0)
                    ready = min((pr0 + nr - 1) // 3, NPAIR + 1)
                    if half == 0 and pr0 + nr > 63:
                        ready = NPAIR + 1  # needs leftover chunk 21
                    store_plan.append(
                        (ready, mk_store(half, pr0, base_yr + pr0, nr)))

            TAPS = [(kh, kw) for kh in range(KS) for kw in range(KS)]

            for c in range(NPAIR):
                c2 = c + 22
                n2 = chunk_n(c2)
                pa = pp.tile([128, 512], f32, tag="ps")
                pb = pp.tile([128, 512], f32, tag="ps")
                for t, (kh, kw) in enumerate(TAPS):
                    st, sp = (t == 0), (t == len(TAPS) - 1)
                    # img0 chunk c -> A[0:64];  img0 chunk c+22 -> A[64:128]
                    nc.tensor.matmul(pa[0:64, 0:CHN], lhsT(0, t),
                                     rhs(0, c, kh, kw, CHN), start=st, stop=sp,
                                     skip_group_check=True)
                    nc.tensor.matmul(pa[64:128, 0:n2], lhsT(0, t),
                                     rhs(0, c2, kh, kw, n2), start=st, stop=sp,
                                     skip_group_check=True)
                    # img1 chunk c -> B[0:64];  img1 chunk c+22 -> B[64:128]
                    nc.tensor.matmul(pb[0:64, 0:CHN], lhsT(1, t),
                                     rhs(1, c, kh, kw, CHN), start=st, stop=sp,
                                     skip_group_check=True)
                    nc.tensor.matmul(pb[64:128, 0:n2], lhsT(1, t),
                                     rhs(1, c2, kh, kw, n2), start=st, stop=sp,
                                     skip_group_check=True)

                # evict PSUM -> stage.  img0 slots at c*CHN, img1 at (22+c)*CHN
                pa3 = pa[:, 0:CHN].rearrange("p (h w) -> p h w", w=WP)
                pb3 = pb[:, 0:CHN].rearrange("p (h w) -> p h w", w=WP)
                nr2 = n2 // WP
                nc.any.tensor_copy(st3[0:64, 3 * c:3 * c + 3, :],
                                   pa3[0:64, 0:3, 0:W])
                nc.any.tensor_copy(st3[64:128, 3 * c:3 * c + nr2, :],
                                   pa3[64:128, 0:nr2, 0:W])
                nc.any.tensor_copy(st3[0:64, 66 + 3 * c:66 + 3 * c + 3, :],
                                   pb3[0:64, 0:3, 0:W])
                nc.any.tensor_copy(st3[64:128, 66 + 3 * c:66 + 3 * c + nr2, :],
                                   pb3[64:128, 0:nr2, 0:W])

                if c == 5:
                    # upper-half crop mask: y rows 66..83 = view rows 0..17
                    # (img0) and 66..83 (img1), written by pairs 0..5
                    for ib in range(2):
                        nc.any.memset(
                            st3[64:128, 66 * ib:66 * ib + 18, CROP0:CROP1], 0.0)
                emit_stores_ready(c)

            # leftover chunk 21 (rows 63-65), both images, via two banks
            pc_ = pp.tile([128, 512], f32, tag="ps")
            pd_ = pp.tile([128, 512], f32, tag="ps")
            for t, (kh, kw) in enumerate(TAPS):
                st, sp = (t == 0), (t == len(TAPS) - 1)
                nc.tensor.matmul(pc_[0:64, 0:CHN], lhsT(0, t),
                                 rhs(0, 21, kh, kw, CHN), start=st, stop=sp,
                                 skip_group_check=True)
                nc.tensor.matmul(pd_[0:64, 0:CHN], lhsT(1, t),
                                 rhs(1, 21, kh, kw, CHN), start=st, stop=sp,
                                 skip_group_check=True)
            pc3 = pc_[:, 0:CHN].rearrange("p (h w) -> p h w", w=WP)
            pd3 = pd_[:, 0:CHN].rearrange("p (h w) -> p h w", w=WP)
            nc.any.tensor_copy(st3[0:64, 63:66, :], pc3[0:64, 0:3, 0:W])
            nc.any.tensor_copy(st3[0:64, 129:132, :], pd3[0:64, 0:3, 0:W])

            # lower-half crop mask: y rows 44..65 = view rows 44..65 (img0)
            # and 110..131 (img1); written by pairs 14..20 + leftover
            for ib in range(2):
                nc.any.memset(
                    st3[0:64, 66 * ib + CROP0:66 * ib + 66, CROP0:CROP1], 0.0)

            emit_stores_ready(NPAIR + 1)
            assert not store_plan, store_plan

    nc.compile()
    return nc


def _get_module():
    if "nc" not in _CACHE:
        _CACHE["nc"] = _build_module()
    return _CACHE["nc"]


def _make_in_maps(x, weight):
    x = np.asarray(x, dtype=np.float32)
    weight = np.asarray(weight, dtype=np.float32)
    # host marshaling: pad x into the row-major stride-129 layout
    xp = np.zeros((B, C, HP, WP), dtype=np.float32)
    xp[:, :, 1:H + 1, 1:W + 1] = x
    xp = xp.reshape(B, C, XLEN)
    import ml_dtypes
    xp = xp.astype(ml_dtypes.bfloat16)
    # weight [oc, ic, kh, kw] -> [ic, (kh kw), oc]
    import ml_dtypes
    wt = np.ascontiguousarray(
        weight.transpose(1, 2, 3, 0).reshape(C, KS * KS, OC)
    ).astype(ml_dtypes.bfloat16)
    return [
        {"xin": np.ascontiguousarray(xp[k * IMGS:(k + 1) * IMGS]), "wt": wt}
        for k in range(N_CORES)
    ]


def kernel(x, weight):
    from concourse.bass_utils import run_bass_kernel_spmd

    nc = _get_module()
    in_maps = _make_in_maps(x, weight)
    res = run_bass_kernel_spmd(nc, in_maps, list(range(N_CORES)))
    out = np.concatenate([res.results[k]["yout"] for k in range(N_CORES)],
                         axis=0)
    return out.astype(np.float32, copy=False)



# revision 2
# speedup vs baseline: 1.5949x; 1.5949x over previous
"""Trainium2 Bass kernel for CropConv: 3x3 same-padding conv (64->64 ch) on
[16, 64, 128, 128] fp32 input, with a static crop mask zeroing output rows/cols
[44:84).

Strategy (data-parallel over batch, 8 cores x 2 images each):
  - Host marshals x into a zero-padded row-major layout with row stride 129
    (131 padded rows), so every conv tap (kh, kw) of an output row-chunk is one
    contiguous rhs slice.  Image 0 in SBUF partitions 0-63 (partition =
    in-channel), image 1 in partitions 64-127.
  - The conv is 9 PSUM-accumulated TensorE matmuls per output chunk; four
    64x64 matmuls run concurrently in the four quadrants of the PE array
    (row-half = image, col-half = chunk pairing (c, c+22)).
  - PSUM -> SBUF stage eviction as a single 128-partition DVE copy per PSUM
    tile with fp32 -> bf16 conversion; crop-mask memsets on GpSimd; bf16
    output stores stream on the scalar HWDGE ring interleaved with compute
    (12-row pieces); input loads stream on the sync ring in 10 prioritized
    segments so compute starts early.  Host upcasts the bf16 output to fp32.
"""

import numpy as np

# ---- problem constants (hardcoded; kernel.py must be self-contained) ----
B, C, H, W = 16, 64, 128, 128
OC, KS = 64, 3
N_CORES = 8
IMGS = B // N_CORES  # 2 images per core

WP = W + 1            # padded row stride: 129
HP = H + 3            # padded rows in the x buffer: 131
XLEN = HP * WP        # 16899 elems per partition

RPC = 3               # output rows per chunk
NCH = (H + RPC - 1) // RPC          # 43 chunks per image (last has 2 rows)
NPAIR = 21            # chunk pairs (c, c+22); chunk 21 is the leftover
CHN = RPC * WP        # matmul free dim per full chunk: 387
BAND = 66             # stage rows per band (band0 = y rows 0-65 in
                      # partitions 0-63, band1 = y rows 66-127 in 64-127)
STLEN = IMGS * BAND * W  # stage free size per partition: 16896

CROP0, CROP1 = 44, 84  # masked rows/cols [44, 84)

_CACHE = {}


def _build_module():
    import concourse.tile as tile
    from concourse import bacc, mybir

    f32 = mybir.dt.float32
    bf16 = mybir.dt.bfloat16

    nc = bacc.Bacc("TRN2", target_bir_lowering=False, debug=False,
                   num_devices=N_CORES)

    x_ap = nc.dram_tensor("xin", [IMGS, C, XLEN], bf16,
                          kind="ExternalInput").ap()
    w_ap = nc.dram_tensor("wt", [2 * C, KS * KS * OC], bf16,
                          kind="ExternalInput").ap()
    y_ap = nc.dram_tensor("yout", [IMGS, OC, H, W], bf16,
                          kind="ExternalOutput").ap()

    x_bc = x_ap.rearrange("b c l -> (b c) l")  # [128, XLEN]

    with tile.TileContext(nc) as tc:
        with tc.tile_pool(name="big", bufs=1) as big, \
             tc.tile_pool(name="psum", bufs=8, space="PSUM") as pp:

            x_sb = big.tile([128, XLEN], bf16, tag="xbuf")
            stage = big.tile([128, STLEN], bf16, tag="stage")
            w_sb = big.tile([128, KS * KS * OC], bf16, tag="wbuf")

            # [p, img, band-row, col]
            st4 = stage.rearrange("p (i h w) -> p i h w", i=IMGS, w=W)

            # weights (pre-duplicated on host into both partition halves)
            nc.sync.dma_start(out=w_sb, in_=w_ap)

            # x loads: contiguous padded-row segments, ordered so the first
            # chunk pairs (rows 0.. and 66..) become computable immediately
            segs = [(0, 9), (65, 74), (9, 21), (74, 86), (21, 33), (86, 99),
                    (33, 49), (99, 115), (49, 65), (115, 131)]
            for (a, b_) in segs:
                nc.sync.dma_start(out=x_sb[:, a * WP:b_ * WP],
                                  in_=x_bc[:, a * WP:b_ * WP])

            def lhsT(half, t):
                return w_sb[half * 64:(half + 1) * 64, t * OC:(t + 1) * OC]

            def rhs(half, c, kh, kw, n):
                off = (RPC * c + kh) * WP + kw
                return x_sb[half * 64:(half + 1) * 64, off:off + n]

            TAPS = [(kh, kw) for kh in range(KS) for kw in range(KS)]

            def store_piece(band, r0, nr):
                # one store per (band, row range): 64 partitions (= oc),
                # free dims (img, rows, cols); dst y rows offset by 66*band
                src = st4[band * 64:band * 64 + 64, :, r0:r0 + nr, :]
                yr0 = BAND * band + r0
                dst = y_ap[:, :, yr0:yr0 + nr, :].rearrange(
                    "b o h w -> o b h w")
                nc.scalar.dma_start(out=dst, in_=src)

            def mask_memset(band, r0, r1):
                for i in range(IMGS):
                    nc.gpsimd.memset(
                        st4[band * 64:band * 64 + 64, i, r0:r1,
                            CROP0:CROP1], 0.0)

            # store piece k covers band rows [12k, 12k+12) of both bands and
            # is ready after pair 4k+3 (band0 chunks 4k..4k+3, band1 chunks
            # 22+4k..22+4k+3).  Masked y rows [44,84) -> per-piece memsets.
            piece_after_pair = {3: 0, 7: 1, 11: 2, 15: 3, 19: 4}
            # (band, r0, r1) memsets due right before each piece's stores
            piece_memsets = {
                0: [(1, 0, 12)],          # y rows 66-77
                1: [(1, 12, 18)],         # y rows 78-83
                3: [(0, 44, 48)],         # y rows 44-47
                4: [(0, 48, 60)],         # y rows 48-59
                5: [(0, 60, 66)],         # y rows 60-65
            }

            for c in range(NPAIR):
                c2 = c + 22
                n2 = 2 * WP if c2 == NCH - 1 else CHN  # 258 for chunk 42
                pa = pp.tile([128, 512], f32, tag="ps")
                pb = pp.tile([128, 512], f32, tag="ps")
                for t, (kh, kw) in enumerate(TAPS):
                    st, sp = (t == 0), (t == len(TAPS) - 1)
                    # img0 chunk c -> A[0:64];  img0 chunk c+22 -> A[64:128]
                    nc.tensor.matmul(pa[0:64, 0:CHN], lhsT(0, t),
                                     rhs(0, c, kh, kw, CHN), start=st, stop=sp,
                                     skip_group_check=True)
                    nc.tensor.matmul(pa[64:128, 0:n2], lhsT(0, t),
                                     rhs(0, c2, kh, kw, n2), start=st, stop=sp,
                                     skip_group_check=True)
                    # img1 chunk c -> B[0:64];  img1 chunk c+22 -> B[64:128]
                    nc.tensor.matmul(pb[0:64, 0:CHN], lhsT(1, t),
                                     rhs(1, c, kh, kw, CHN), start=st, stop=sp,
                                     skip_group_check=True)
                    nc.tensor.matmul(pb[64:128, 0:n2], lhsT(1, t),
                                     rhs(1, c2, kh, kw, n2), start=st, stop=sp,
                                     skip_group_check=True)

                # evict PSUM -> stage: band0 rows 3c..3c+2 (partitions 0-63)
                # and band1 rows 3c..3c+2 (partitions 64-127) share the same
                # free offset -> one 128-partition DVE copy per PSUM tile
                pa3 = pa[:, 0:CHN].rearrange("p (h w) -> p h w", w=WP)
                pb3 = pb[:, 0:CHN].rearrange("p (h w) -> p h w", w=WP)
                if c < NPAIR - 1:
                    nc.vector.tensor_copy(st4[:, 0, 3 * c:3 * c + 3, :],
                                          pa3[:, 0:3, 0:W])
                    nc.vector.tensor_copy(st4[:, 1, 3 * c:3 * c + 3, :],
                                          pb3[:, 0:3, 0:W])
                else:
                    # chunk 42 has only 2 rows -> split the last eviction
                    for i, p3 in ((0, pa3), (1, pb3)):
                        nc.vector.tensor_copy(
                            st4[0:64, i, 3 * c:3 * c + 3, :],
                            p3[0:64, 0:3, 0:W])
                        nc.vector.tensor_copy(
                            st4[64:128, i, 3 * c:3 * c + 2, :],
                            p3[64:128, 0:2, 0:W])

                if c in piece_after_pair:
                    k = piece_after_pair[c]
                    for (band, r0, r1) in piece_memsets.get(k, []):
                        mask_memset(band, r0, r1)
                    store_piece(0, 12 * k, 12)
                    store_piece(1, 12 * k, 12)

            # leftover chunk 21 (y rows 63-65), both images, via two banks
            pc_ = pp.tile([128, 512], f32, tag="ps")
            pd_ = pp.tile([128, 512], f32, tag="ps")
            for t, (kh, kw) in enumerate(TAPS):
                st, sp = (t == 0), (t == len(TAPS) - 1)
                nc.tensor.matmul(pc_[0:64, 0:CHN], lhsT(0, t),
                                 rhs(0, 21, kh, kw, CHN), start=st, stop=sp,
                                 skip_group_check=True)
                nc.tensor.matmul(pd_[0:64, 0:CHN], lhsT(1, t),
                                 rhs(1, 21, kh, kw, CHN), start=st, stop=sp,
                                 skip_group_check=True)
            pc3 = pc_[:, 0:CHN].rearrange("p (h w) -> p h w", w=WP)
            pd3 = pd_[:, 0:CHN].rearrange("p (h w) -> p h w", w=WP)
            nc.vector.tensor_copy(st4[0:64, 0, 63:66, :], pc3[0:64, 0:3, 0:W])
            nc.vector.tensor_copy(st4[0:64, 1, 63:66, :], pd3[0:64, 0:3, 0:W])

            # final piece: band0 rows 60-65 (chunks 20, 21), band1 rows
            # 126-127 (chunk 42)
            for (band, r0, r1) in piece_memsets[5]:
                mask_memset(band, r0, r1)
            store_piece(0, 60, 6)
            store_piece(1, 60, 2)

    nc.compile()
    return nc


def _get_module():
    if "nc" not in _CACHE:
        _CACHE["nc"] = _build_module()
    return _CACHE["nc"]


def _make_in_maps(x, weight):
    x = np.asarray(x, dtype=np.float32)
    weight = np.asarray(weight, dtype=np.float32)
    # host marshaling: pad x into the row-major stride-129 layout
    xp = np.zeros((B, C, HP, WP), dtype=np.float32)
    xp[:, :, 1:H + 1, 1:W + 1] = x
    xp = xp.reshape(B, C, XLEN)
    import ml_dtypes
    xp = xp.astype(ml_dtypes.bfloat16)
    # weight [oc, ic, kh, kw] -> [ic, (kh kw), oc], duplicated into both
    # partition halves so a single 128-partition DMA loads it
    wt = np.ascontiguousarray(
        weight.transpose(1, 2, 3, 0).reshape(C, KS * KS * OC)
    ).astype(ml_dtypes.bfloat16)
    wt2 = np.concatenate([wt, wt], axis=0)
    return [
        {"xin": np.ascontiguousarray(xp[k * IMGS:(k + 1) * IMGS]), "wt": wt2}
        for k in range(N_CORES)
    ]


def kernel(x, weight):
    from concourse.bass_utils import run_bass_kernel_spmd

    nc = _get_module()
    in_maps = _make_in_maps(x, weight)
    res = run_bass_kernel_spmd(nc, in_maps, list(range(N_CORES)))
    out = np.concatenate([res.results[k]["yout"] for k in range(N_CORES)],
                         axis=0)
    return out.astype(np.float32)


# revision 6
# speedup vs baseline: 1.6207x; 1.0162x over previous
"""Trainium2 Bass kernel for CropConv: 3x3 same-padding conv (64->64 ch) on
[16, 64, 128, 128] fp32 input, with a static crop mask zeroing output rows/cols
[44:84).

Strategy (data-parallel over batch, 8 cores x 2 images each):
  - Host marshals x into a zero-padded row-major layout with row stride 129
    (131 padded rows), so every conv tap (kh, kw) of an output row-chunk is one
    contiguous rhs slice.  Image 0 in SBUF partitions 0-63 (partition =
    in-channel), image 1 in partitions 64-127.
  - The conv is 9 PSUM-accumulated TensorE matmuls per output chunk; four
    64x64 matmuls run concurrently in the four quadrants of the PE array
    (row-half = image, col-half = chunk pairing (c, c+22)).
  - PSUM -> SBUF stage eviction as a single 128-partition DVE copy per PSUM
    tile with fp32 -> bf16 conversion; crop-mask memsets on GpSimd; bf16
    output stores stream on the scalar HWDGE ring interleaved with compute
    (12-row pieces); input loads stream on the sync ring in 10 prioritized
    segments so compute starts early.  Host upcasts the bf16 output to fp32.
"""

import numpy as np

# ---- problem constants (hardcoded; kernel.py must be self-contained) ----
B, C, H, W = 16, 64, 128, 128
OC, KS = 64, 3
N_CORES = 8
IMGS = B // N_CORES  # 2 images per core

WP = W + 1            # padded row stride: 129
HP = H + 3            # padded rows in the x buffer: 131
XLEN = HP * WP        # 16899 elems per partition

RPC = 3               # output rows per chunk
NCH = (H + RPC - 1) // RPC          # 43 chunks per image (last has 2 rows)
NPAIR = 21            # chunk pairs (c, c+22); chunk 21 is the leftover
CHN = RPC * WP        # matmul free dim per full chunk: 387
BAND = 66             # stage rows per band (band0 = y rows 0-65 in
                      # partitions 0-63, band1 = y rows 66-127 in 64-127)
STLEN = IMGS * BAND * W  # stage free size per partition: 16896

CROP0, CROP1 = 44, 84  # masked rows/cols [44, 84)

_CACHE = {}


def _build_module():
    import concourse.tile as tile
    from concourse import bacc, mybir

    f32 = mybir.dt.float32
    bf16 = mybir.dt.bfloat16

    nc = bacc.Bacc("TRN2", target_bir_lowering=False, debug=False,
                   num_devices=N_CORES)

    x_ap = nc.dram_tensor("xin", [IMGS, C, XLEN], bf16,
                          kind="ExternalInput").ap()
    w_ap = nc.dram_tensor("wt", [2 * C, KS * KS * OC], bf16,
                          kind="ExternalInput").ap()
    y_ap = nc.dram_tensor("yout", [IMGS, OC, H, W], bf16,
                          kind="ExternalOutput").ap()

    x_bc = x_ap.rearrange("b c l -> (b c) l")  # [128, XLEN]

    with tile.TileContext(nc) as tc:
        with tc.tile_pool(name="big", bufs=1) as big, \
             tc.tile_pool(name="psum", bufs=8, space="PSUM") as pp:

            x_sb = big.tile([128, XLEN], bf16, tag="xbuf")
            stage = big.tile([128, STLEN], bf16, tag="stage")
            w_sb = big.tile([128, KS * KS * OC], bf16, tag="wbuf")
            scr = big.tile([128, 512], bf16, tag="scratch")

            # [p, img, band-row, col]
            st4 = stage.rearrange("p (i h w) -> p i h w", i=IMGS, w=W)

            # weights (pre-duplicated on host into both partition halves) on
            # the scalar HWDGE ring, concurrent with the first x loads below
            nc.scalar.dma_start(out=w_sb, in_=w_ap)

            # x loads: contiguous padded-row segments, ordered so the first
            # chunk pairs (rows 0.. and 66..) become computable immediately
            segs = [(0, 9), (65, 74), (9, 21), (74, 86), (21, 33), (86, 99),
                    (33, 49), (99, 115), (49, 65), (115, 131)]
            for (a, b_) in segs:
                nc.sync.dma_start(out=x_sb[:, a * WP:b_ * WP],
                                  in_=x_bc[:, a * WP:b_ * WP])

            # PE warm-up: dummy matmuls on scratch while the first x segments
            # stream in, so HAM un-throttles (K=8/8) before the real matmuls
            # start and the conv runs warm from pair 0.
            nc.gpsimd.memset(scr[:, :], 0.0)
            pw = pp.tile([128, 512], f32, tag="ps")
            for _ in range(12):
                nc.tensor.matmul(pw[:, 0:512], scr[:, 0:128], scr[:, 0:512],
                                 start=True, stop=True, skip_group_check=True)

            def lhsT(half, t):
                return w_sb[half * 64:(half + 1) * 64, t * OC:(t + 1) * OC]

            def rhs(half, c, kh, kw, n):
                off = (RPC * c + kh) * WP + kw
                return x_sb[half * 64:(half + 1) * 64, off:off + n]

            TAPS = [(kh, kw) for kh in range(KS) for kw in range(KS)]

            def store_piece(band, r0, nr, eng):
                # one store per (band, row range): 64 partitions (= oc),
                # free dims (img, rows, cols); dst y rows offset by 66*band
                src = st4[band * 64:band * 64 + 64, :, r0:r0 + nr, :]
                yr0 = BAND * band + r0
                dst = y_ap[:, :, yr0:yr0 + nr, :].rearrange(
                    "b o h w -> o b h w")
                eng.dma_start(out=dst, in_=src)

            def mask_memset(band, r0, r1):
                for i in range(IMGS):
                    nc.gpsimd.memset(
                        st4[band * 64:band * 64 + 64, i, r0:r1,
                            CROP0:CROP1], 0.0)

            # store piece k covers band rows [6k, 6k+6) of both bands and is
            # ready after pair 2k+1 (band0 chunks 2k..2k+1, band1 chunks
            # 22+2k..23+2k).  Masked y rows [44,84) -> per-piece memsets.
            # band0 stores ride the scalar ring, band1 the sync ring.
            piece_after_pair = {2 * k + 1: k for k in range(10)}
            # (band, r0, r1) memsets due right before each piece's stores
            piece_memsets = {
                0: [(1, 0, 6)],           # y rows 66-71
                1: [(1, 6, 12)],          # y rows 72-77
                2: [(1, 12, 18)],         # y rows 78-83
                7: [(0, 44, 48)],         # y rows 44-47
                8: [(0, 48, 54)],         # y rows 48-53
                9: [(0, 54, 60)],         # y rows 54-59
                10: [(0, 60, 66)],        # y rows 60-65
            }

            for c in range(NPAIR):
                c2 = c + 22
                n2 = 2 * WP if c2 == NCH - 1 else CHN  # 258 for chunk 42
                pa = pp.tile([128, 512], f32, tag="ps")
                pb = pp.tile([128, 512], f32, tag="ps")
                for t, (kh, kw) in enumerate(TAPS):
                    st, sp = (t == 0), (t == len(TAPS) - 1)
                    # img0 chunk c -> A[0:64];  img0 chunk c+22 -> A[64:128]
                    nc.tensor.matmul(pa[0:64, 0:CHN], lhsT(0, t),
                                     rhs(0, c, kh, kw, CHN), start=st, stop=sp,
                                     skip_group_check=True)
                    nc.tensor.matmul(pa[64:128, 0:n2], lhsT(0, t),
                                     rhs(0, c2, kh, kw, n2), start=st, stop=sp,
                                     skip_group_check=True)
                    # img1 chunk c -> B[0:64];  img1 chunk c+22 -> B[64:128]
                    nc.tensor.matmul(pb[0:64, 0:CHN], lhsT(1, t),
                                     rhs(1, c, kh, kw, CHN), start=st, stop=sp,
                                     skip_group_check=True)
                    nc.tensor.matmul(pb[64:128, 0:n2], lhsT(1, t),
                                     rhs(1, c2, kh, kw, n2), start=st, stop=sp,
                                     skip_group_check=True)

                # evict PSUM -> stage: band0 rows 3c..3c+2 (partitions 0-63)
                # and band1 rows 3c..3c+2 (partitions 64-127) share the same
                # free offset -> one 128-partition DVE copy per PSUM tile
                pa3 = pa[:, 0:CHN].rearrange("p (h w) -> p h w", w=WP)
                pb3 = pb[:, 0:CHN].rearrange("p (h w) -> p h w", w=WP)
                if c < NPAIR - 1:
                    nc.vector.tensor_copy(st4[:, 0, 3 * c:3 * c + 3, :],
                                          pa3[:, 0:3, 0:W])
                    nc.vector.tensor_copy(st4[:, 1, 3 * c:3 * c + 3, :],
                                          pb3[:, 0:3, 0:W])
                else:
                    # chunk 42 has only 2 rows -> split the last eviction
                    for i, p3 in ((0, pa3), (1, pb3)):
                        nc.vector.tensor_copy(
                            st4[0:64, i, 3 * c:3 * c + 3, :],
                            p3[0:64, 0:3, 0:W])
                        nc.vector.tensor_copy(
                            st4[64:128, i, 3 * c:3 * c + 2, :],
                            p3[64:128, 0:2, 0:W])

                if c in piece_after_pair:
                    k = piece_after_pair[c]
                    for (band, r0, r1) in piece_memsets.get(k, []):
                        mask_memset(band, r0, r1)
                    store_piece(0, 6 * k, 6, nc.scalar)
                    store_piece(1, 6 * k, 6, nc.sync)

            # leftover chunk 21 (y rows 63-65), both images, via two banks
            pc_ = pp.tile([128, 512], f32, tag="ps")
            pd_ = pp.tile([128, 512], f32, tag="ps")
            for t, (kh, kw) in enumerate(TAPS):
                st, sp = (t == 0), (t == len(TAPS) - 1)
                nc.tensor.matmul(pc_[0:64, 0:CHN], lhsT(0, t),
                                 rhs(0, 21, kh, kw, CHN), start=st, stop=sp,
                                 skip_group_check=True)
                nc.tensor.matmul(pd_[0:64, 0:CHN], lhsT(1, t),
                                 rhs(1, 21, kh, kw, CHN), start=st, stop=sp,
                                 skip_group_check=True)
            pc3 = pc_[:, 0:CHN].rearrange("p (h w) -> p h w", w=WP)
            pd3 = pd_[:, 0:CHN].rearrange("p (h w) -> p h w", w=WP)
            nc.vector.tensor_copy(st4[0:64, 0, 63:66, :], pc3[0:64, 0:3, 0:W])
            nc.vector.tensor_copy(st4[0:64, 1, 63:66, :], pd3[0:64, 0:3, 0:W])

            # final piece: band0 rows 60-65 (chunks 20, 21), band1 rows
            # 126-127 (chunk 42)
            for (band, r0, r1) in piece_memsets[10]:
                mask_memset(band, r0, r1)
            store_piece(0, 60, 6, nc.scalar)
            store_piece(1, 60, 2, nc.sync)

    nc.compile()
    return nc


def _get_module():
    if "nc" not in _CACHE:
        _CACHE["nc"] = _build_module()
    return _CACHE["nc"]


def _make_in_maps(x, weight):
    x = np.asarray(x, dtype=np.float32)
    weight = np.asarray(weight, dtype=np.float32)
    # host marshaling: pad x into the row-major stride-129 layout
    xp = np.zeros((B, C, HP, WP), dtype=np.float32)
    xp[:, :, 1:H + 1, 1:W + 1] = x
    xp = xp.reshape(B, C, XLEN)
    import ml_dtypes
    xp = xp.astype(ml_dtypes.bfloat16)
    # weight [oc, ic, kh, kw] -> [ic, (kh kw), oc], duplicated into both
    # partition halves so a single 128-partition DMA loads it
    wt = np.ascontiguousarray(
        weight.transpose(1, 2, 3, 0).reshape(C, KS * KS * OC)
    ).astype(ml_dtypes.bfloat16)
    wt2 = np.concatenate([wt, wt], axis=0)
    return [
        {"xin": np.ascontiguousarray(xp[k * IMGS:(k + 1) * IMGS]), "wt": wt2}
        for k in range(N_CORES)
    ]


def kernel(x, weight):
    from concourse.bass_utils import run_bass_kernel_spmd

    nc = _get_module()
    in_maps = _make_in_maps(x, weight)
    res = run_bass_kernel_spmd(nc, in_maps, list(range(N_CORES)))
    out = np.concatenate([res.results[k]["yout"] for k in range(N_CORES)],
                         axis=0)
    return out.astype(np.float32)
